# revision 1
# baseline (speedup 1.0000x reference)
"""AFNO spectral attention kernel for 8 Trainium2 NeuronCores.

Math reduction (verified to rel err ~2e-7 against the jax reference):
  The reference does rfft2 -> per-(h-freq, w-mode<8) block-diag channel
  matmul (x sigmoid(gate)) on the first 8 W-modes -> irfft2 -> residual
  -> output projection.  Because the block matmul acts pointwise in the
  H-frequency axis, the H-axis FFTs cancel (F^H F = I), and replacing
  only the first 8 W-modes is equivalent to adding a W-axis low-pass
  filtered correction:

    xlp   = x  (low-pass along w: M = irfft(keep8(rfft(I))))   [64x64, symmetric]
    delta = xlp @ (A_bd - I)        A_bd = blockdiag(sigmoid(g_b) * W_b)
    y     = (x + delta) @ (I + rescale * W_out^T) + rescale * b_out
            (+ a batch-independent bias-image term; zero for these inputs)

  All matmuls run in fp16 on the TensorEngine (1 cycle/row; fp32 is 4x
  slower), accumulating in fp32 PSUM.

Device layout per core (2 of 16 batch images, data-parallel):
  rows r = img*4096 + h*64 + w  ->  16 groups of 512 rows, subtiles of 128
  (= 2 h-rows, so the w-filter is blockdiag(M, M) acting inside a subtile).

  Per group g, per channel-chunk k (6 x 128 channels):
    comb[k,j] = xn_j[:, chunk k].T @ [I | F]    -> [xT-half | xlpT-half] in PSUM
                (ScalarE evacuates xT, VectorE evacuates xlpT, both to fp16)
    pxs[k]    = sum_ki (A_bd - I)[ki,k].T @ xlpT[ki]   banded 128x128 matmuls
                (off-diagonal tile_position matmuls crash the device, so the
                 block-diagonal matmul runs as full-array chunk-pair bands)
    xsT[k]    = fp16(xT[k] + pxs[k])                   VectorE add
  then per row-subtile j:
    y[j]      = sum_k xsT[k][:,j].T @ Wf[k]      Wf = I + rescale*W_out^T
  Residual and projection are fused into Wf; nonzero biases would be folded
  in as a batch-independent host-side additive correction (zero here).
"""

import numpy as np

import concourse.mybir as mybir
import concourse.tile as tile
from concourse import bacc
from concourse.bass_utils import run_bass_kernel_spmd

B, N_TOK, C = 16, 4096, 768
H, W = 64, 64
NB, BS, MODES = 8, 96, 8
NCORES = 8
B_PER = B // NCORES          # 2 images per core
ROWS = B_PER * N_TOK         # 8192 rows per core
GROUP = 512                  # rows per group
NGROUPS = ROWS // GROUP      # 16
NSUB = GROUP // 128          # 4 subtiles of 128 rows
NCHUNK = C // 128            # 6 channel chunks
NSLICE = C // 32             # 24 32-channel slices

DT = mybir.dt.float16
NPDT = np.float16
f32 = mybir.dt.float32


def _filter_matrix():
    """M[w_in, w_out]: keep first MODES rfft modes along w (ortho norm)."""
    eye = np.eye(W)
    fw = np.fft.rfft(eye, axis=1, norm="ortho")
    fw[:, MODES:] = 0
    return np.fft.irfft(fw, n=W, axis=1, norm="ortho")  # symmetric


def _build_consts(block_W, block_b, gates, W_out, b_out, rescale):
    g = 1.0 / (1.0 + np.exp(-gates.astype(np.float64)))
    m64 = _filter_matrix()

    fid = np.zeros((128, 256), dtype=np.float64)
    fid[:, 0:128] = np.eye(128)
    fid[0:64, 128:192] = m64
    fid[64:128, 192:256] = m64

    # (A_bd - I) as full 128x128 chunk-pair bands of the 768x768 block-diagonal
    # matrix (off-diagonal tile_position matmuls crash the device, so the
    # block matmul runs as full-array banded matmuls instead).
    ami = g[:, None, None] * block_W.astype(np.float64) - np.eye(BS)[None]
    gmat = np.zeros((C, C), dtype=np.float64)
    for b_ in range(NB):
        gmat[BS * b_ : BS * (b_ + 1), BS * b_ : BS * (b_ + 1)] = ami[b_]
    asub = np.zeros((128, 128 * len(GPAIRS)), dtype=np.float64)
    for idx, (ki, ko) in enumerate(GPAIRS):
        asub[:, 128 * idx : 128 * (idx + 1)] = gmat[
            128 * ki : 128 * (ki + 1), 128 * ko : 128 * (ko + 1)
        ]

    wfmat = float(rescale) * W_out.astype(np.float64).T + np.eye(C)
    wf = np.zeros((128, NCHUNK * C), dtype=np.float64)
    for k in range(NCHUNK):
        wf[:, C * k : C * (k + 1)] = wfmat[128 * k : 128 * (k + 1), :]

    return fid.astype(NPDT), asub.astype(NPDT), wf.astype(NPDT)


def _bias_correction(block_b, gates, W_out, b_out, rescale):
    """Batch-independent additive output term from the biases (zero for the
    standard inputs).  The frequency-domain bias g_b*block_b[d]*(1+1j) sits at
    every (h-freq, mode<MODES); its irfft2 image adds to xs before the output
    projection, plus rescale*b_out after it."""
    if not (np.any(block_b) or np.any(b_out)):
        return None
    g = 1.0 / (1.0 + np.exp(-gates.astype(np.float64)))
    bias_freq = np.zeros((NB, BS, H, W // 2 + 1), dtype=np.complex128)
    bb = (g[:, None] * block_b.astype(np.float64))[:, :, None]
    bias_freq[:, :, :, :MODES] = np.broadcast_to(
        bb[:, :, :, None] * (1.0 + 1.0j), (NB, BS, H, MODES)
    )
    bias_img = np.fft.irfft2(bias_freq, s=(H, W), norm="ortho")  # [NB,BS,H,W]
    bias_img = bias_img.reshape(C, H * W).T  # [N_TOK, C]
    wfmat = float(rescale) * W_out.astype(np.float64).T + np.eye(C)
    corr = bias_img @ wfmat + float(rescale) * b_out.astype(np.float64)[None, :]
    return corr.astype(np.float32)  # [N_TOK, C]


def _gpairs():
    """Nonzero 128x128 chunk pairs (ki, ko) of the block-diagonal matrix."""
    pairs = []
    for ki in range(NCHUNK):
        for ko in range(NCHUNK):
            lo = max(128 * ki, 128 * ko)
            # overlap exists iff some block's [96b, 96b+96) x same square
            # intersects the (ki, ko) chunk rectangle
            hit = any(
                96 * b_ < 128 * (ki + 1)
                and 96 * (b_ + 1) > 128 * ki
                and 96 * b_ < 128 * (ko + 1)
                and 96 * (b_ + 1) > 128 * ko
                for b_ in range(NB)
            )
            if hit:
                pairs.append((ki, ko))
    return pairs


GPAIRS = _gpairs()


def _build_kernel():
    nc = bacc.Bacc("TRN2", target_bir_lowering=False, debug=False, num_devices=NCORES)
    x_ext = nc.declare_dram_parameter("x", [ROWS, C], f32, isOutput=False)
    fid_ext = nc.declare_dram_parameter("fid", [128, 256], DT, isOutput=False)
    asub_ext = nc.declare_dram_parameter(
        "asub", [128, 128 * len(GPAIRS)], DT, isOutput=False
    )
    wf_ext = nc.declare_dram_parameter("wf", [128, NCHUNK * C], DT, isOutput=False)
    out_ext = nc.declare_dram_parameter("out", [ROWS, C], f32, isOutput=True)

    with tile.TileContext(nc) as tc:
        with (
            tc.tile_pool(name="const", bufs=1) as cpool,
            tc.tile_pool(name="io", bufs=3) as iopool,
            tc.tile_pool(name="work", bufs=2) as wpool,
            tc.tile_pool(name="ps_lp", bufs=2, space="PSUM") as ps_lp,
            tc.tile_pool(name="ps_xs", bufs=2, space="PSUM") as ps_xs,
            tc.tile_pool(name="ps_y", bufs=2, space="PSUM") as ps_y,
        ):
            fid = cpool.tile([128, 256], DT)
            nc.sync.dma_start(fid[:], fid_ext[:])
            asub = cpool.tile([128, 128 * len(GPAIRS)], DT)
            nc.sync.dma_start(asub[:], asub_ext[:])
            wf = cpool.tile([128, NCHUNK * C], DT)
            nc.sync.dma_start(wf[:], wf_ext[:])

            for gidx in range(NGROUPS):
                r0 = gidx * GROUP
                # -- load + cast fp32 -> fp16 during DMA (SWDGE), per subtile
                xn = []
                for j in range(NSUB):
                    xj = iopool.tile([128, C], DT, tag=f"xn{j}")
                    nc.gpsimd.dma_start(
                        xj[:], x_ext[r0 + 128 * j : r0 + 128 * (j + 1), :]
                    )
                    xn.append(xj)


                # -- combined transposes: one matmul per (k, j) with rhs [I|F]
                #    produces [xT | xlpT] halves; ACT evacuates xT, DVE xlp.
                xlp_sb = []
                xt_sb = []
                for k in range(NCHUNK):
                    xlp = wpool.tile([128, GROUP], DT, tag=f"xlp{k}")
                    xt = wpool.tile([128, GROUP], DT, tag=f"xt{k}")
                    for jp in range(NSUB // 2):
                        pcomb = ps_lp.tile([128, 2, 256], f32)
                        for jj in range(2):
                            j = 2 * jp + jj
                            nc.tensor.matmul(
                                pcomb[:, jj, :],
                                xn[j][:, 128 * k : 128 * k + 128],
                                fid[:, 0:256],
                                start=(jj == 0),
                                stop=(jj == 1),
                            )
                        nc.scalar.copy(
                            xt[:, 256 * jp : 256 * jp + 256], pcomb[:, :, 0:128]
                        )
                        nc.vector.tensor_copy(
                            xlp[:, 256 * jp : 256 * jp + 256], pcomb[:, :, 128:256]
                        )
                    xlp_sb.append(xlp)
                    xt_sb.append(xt)


                # -- xsT psum: block-diag (A-I)^T sub-matmuls + identity transpose
                xs_sb = []
                for k in range(NCHUNK):
                    xs = wpool.tile([128, GROUP], DT, tag=f"xs{k}")
                    kis = [ki for (ki, ko) in GPAIRS if ko == k]
                    for h in range(GROUP // 512):
                        pxs = ps_xs.tile([128, 512], f32)
                        # deltaT: banded full-array matmuls of (A_bd - I)
                        for n_, ki in enumerate(kis):
                            idx = GPAIRS.index((ki, k))
                            nc.tensor.matmul(
                                pxs[:],
                                asub[:, 128 * idx : 128 * (idx + 1)],
                                xlp_sb[ki][:, 512 * h : 512 * (h + 1)],
                                start=(n_ == 0),
                                stop=(n_ == len(kis) - 1),
                            )
                        # xsT = xT + deltaT
                        nc.vector.tensor_add(
                            xs[:, 512 * h : 512 * (h + 1)],
                            xt_sb[k][:, 512 * h : 512 * (h + 1)],
                            pxs[:],
                        )
                    xs_sb.append(xs)


                # -- final projection (residual folded into Wf)
                for j in range(NSUB):
                    ysj = iopool.tile([128, C], f32, tag=f"ys{j}")
                    py = ps_y.tile([128, C], f32)
                    for k in range(NCHUNK):
                        lhs = xs_sb[k][:, 128 * j : 128 * j + 128]
                        nc.tensor.matmul(
                            py[:, 0:512],
                            lhs,
                            wf[:, C * k : C * k + 512],
                            start=(k == 0),
                            stop=(k == NCHUNK - 1),
                        )
                        nc.tensor.matmul(
                            py[:, 512:C],
                            lhs,
                            wf[:, C * k + 512 : C * (k + 1)],
                            start=(k == 0),
                            stop=(k == NCHUNK - 1),
                        )
                    nc.scalar.copy(ysj[:], py[:])
                    # -- plain fp32 store (HWDGE; no Q7 descriptor generation)
                    nc.sync.dma_start(
                        out_ext[r0 + 128 * j : r0 + 128 * (j + 1), :], ysj[:]
                    )
    nc.compile()
    return nc


_CACHED_NC = None


def _get_nc():
    global _CACHED_NC
    if _CACHED_NC is None:
        _CACHED_NC = _build_kernel()
    return _CACHED_NC


def _run(inputs, trace=False):
    x = np.ascontiguousarray(np.asarray(inputs["x"], dtype=np.float32))
    fid, asub, wf = _build_consts(
        np.asarray(inputs["block_W"], dtype=np.float32),
        np.asarray(inputs["block_b"], dtype=np.float32),
        np.asarray(inputs["gates"], dtype=np.float32),
        np.asarray(inputs["W_out"], dtype=np.float32),
        np.asarray(inputs["b_out"], dtype=np.float32),
        np.asarray(inputs["rescale"], dtype=np.float32),
    )
    corr = _bias_correction(
        np.asarray(inputs["block_b"], dtype=np.float32),
        np.asarray(inputs["gates"], dtype=np.float32),
        np.asarray(inputs["W_out"], dtype=np.float32),
        np.asarray(inputs["b_out"], dtype=np.float32),
        np.asarray(inputs["rescale"], dtype=np.float32),
    )

    nc = _get_nc()
    in_maps = []
    for i in range(NCORES):
        shard = x[i * B_PER : (i + 1) * B_PER].reshape(ROWS, C)
        in_maps.append({"x": shard, "fid": fid, "asub": asub, "wf": wf})
    res = run_bass_kernel_spmd(
        nc, in_maps, core_ids=list(range(NCORES)), trace=trace
    )
    out = np.empty((B, N_TOK, C), dtype=np.float32)
    for i in range(NCORES):
        out[i * B_PER : (i + 1) * B_PER] = res.results[i]["out"].reshape(
            B_PER, N_TOK, C
        )
    if corr is not None:
        out += corr[None, :, :]
    return out, res.exec_time_ns


def kernel(**inputs) -> np.ndarray:
    out, _ = _run(inputs, trace=False)
    return out

